# revision 33
# baseline (speedup 1.0000x reference)
"""Trainium2 Bass kernel for nn_AIO_DownsampleCouplingBlock.

Reference computation (B=32, C=96, H=W=64, split 48/48):
  x1, x2 = x[:, :48], x[:, 48:]
  y2 = down(x2);  a1 = conv3x3_s2(x1, w_hi) + b_hi
  y2 = y2 * exp(2*tanh(0.2*a1[:192])) + a1[192:]
  y1 = down(x1);  a2 = conv3x3_s1(y2, w_lo) + b_lo
  y1 = y1 * exp(2*tanh(0.2*a2[:192])) + a2[192:]
  out = perm_w @ (concat(y1, y2) * scale + offset)   (channel matmul)
  scale = 0.2*softplus(0.5*act_norm), offset = act_offset

Data-parallel over batch: 4 samples per core on 8 cores.  Everything static
(actnorm folding, channel reorders, permutation indices) is precomputed on
host in numpy.  On device:
  - conv_hi: 4 bf16 matmuls per output chunk.  x1hp [128, 33, 66] holds even
    half-rows at partitions 0-47, odd at 64-111; the 48 "lone" rows
    (ki=0,kj=2) are stuffed into the gaps: partitions 48-63 / 112-127 hold
    row-shifted odd-plane replicas for 32 channels (weights live only in the
    kj=2 tap, which reads K=128), and the SK tile's gap 48-63 holds the last
    16 (built by one extra on-device copy).  Vertical taps ki=1,2 merge into
    one matmul per kj; ki=0 kj=0,1 merge via the column-shifted SK copy.
  - conv_lo: 15 matmuls per output chunk (vs 18 naive).  sigma=0 taps read
    the two 96-partition y2 tiles directly; sigma=1,2 read three dense
    K=128 tiles T1/T2/T3 built per sample by contiguous engine copies of y2
    (T3 mixes sigma by storing its sigma=1 rows pre-shifted one column).
  - conv output rows are reordered (host) so every PSUM->SBUF eviction piece
    starts at partition 0/32/64/96 (hardware window rule):
      [E0(96), T0(0:32) | T0(32:96), E1(0:64) | T1(96), E1(64:96)]
  - epilogue: tanh on ScalarE (scale=0.2), exp on ScalarE with ln(actnorm
    scale) folded into the bias, mul on VectorE, add on VectorE reading the
    conv t-half directly from PSUM.
  - actnorm scale is folded into: exp bias (for the multiplicative half),
    conv t-half weight rows, and 1/scale into conv_lo's input-channel
    weights (y2 is stored pre-scaled).
  - the channel permutation + output writes are indirect-DMA scatters of the
    (bf16) y tiles straight to the f32 DRAM output, one 96-row scatter per
    (tile, sample).
  - the tensor queue is software-pipelined: conv_hi(s+1) + epi1(s+1) are
    emitted before conv_lo(s), so the epi1 y2 chain always has a full
    conv_lo of cover and the PE never waits on the epilogue.
"""
import sys, os
sys.path.insert(0, '/opt/trn_rl_repo')
import numpy as np
import ml_dtypes

import concourse.bass as bass
import concourse.mybir as mybir
from concourse.tile import TileContext
from concourse.bass_utils import run_bass_kernel_spmd

F32 = mybir.dt.float32
BF16 = mybir.dt.bfloat16
I32 = mybir.dt.int32
AF = mybir.ActivationFunctionType
MUL = mybir.AluOpType.mult
ADD = mybir.AluOpType.add

N_CORES = 8
B = 32
SPS = B // N_CORES            # samples per core
NCHUNK = 512                  # matmul free size (16 rows x 32 cols)

# stored partition p (0..95) of a (dj) tile -> down-channel k = 4c + 2di + dj
def _down_idx(p, dj):
    di, c = divmod(p, 48)
    return 4 * c + 2 * di + dj

def _psum_rows():
    e0 = [_down_idx(p, 0) for p in range(96)]
    e1 = [_down_idx(p, 1) for p in range(96)]
    t0 = [192 + _down_idx(p, 0) for p in range(96)]
    t1 = [192 + _down_idx(p, 1) for p in range(96)]
    return np.array(e0 + t0[:32] + t0[32:] + e1[:64] + t1 + e1[64:], np.int64)

_ROWS = _psum_rows()

def _bf(a):
    return np.ascontiguousarray(a).astype(ml_dtypes.bfloat16)


# ---------------------------------------------------------------------------
# host-side preprocessing
# ---------------------------------------------------------------------------
def _prepare(x, w_hi, b_hi, w_lo, b_lo, act_norm, act_offset, perm_w):
    x = np.asarray(x, np.float32)
    w_hi = np.asarray(w_hi, np.float32); w_lo = np.asarray(w_lo, np.float32)
    b_hi = np.asarray(b_hi, np.float32); b_lo = np.asarray(b_lo, np.float32)
    act_norm = np.asarray(act_norm, np.float32).reshape(-1)
    act_offset = np.asarray(act_offset, np.float32).reshape(-1)
    perm_w = np.asarray(perm_w, np.float32)

    scale = 0.2 * np.log1p(np.exp(0.5 * act_norm))          # softplus, beta=0.5
    assert np.allclose(b_hi, 0) and np.allclose(b_lo, 0) and \
        np.allclose(act_offset, 0), "nonzero conv bias / actnorm offset not implemented"
    scale1, scale2 = scale[:192], scale[192:]

    # ---- input layouts (bf16) ----
    def halfrows(xc):                      # xc [B, 48, 64, 64] -> [B, 96, 32, 64]
        v = xc.reshape(B, 48, 32, 2, 64).transpose(0, 3, 1, 2, 4)
        return v.reshape(B, 96, 32, 64)
    x1h = _bf(halfrows(x[:, :48]))
    x2h = _bf(halfrows(x[:, 48:]))
    # x1hp [B, 128, 33, 66]: 0:48 even half-rows, 64:112 odd half-rows,
    # 48:64 / 112:128 = odd planes shifted down one row (lone-tap replicas,
    # channels 0:16 / 16:32), read only by the K=128 kj=2 tap.
    x1hp = np.zeros((B, 128, 33, 66), np.float32)
    x1hp[:, 0:48, 1:33, 1:65] = x[:, :48, 0::2, :]      # di=0: i_pad = x-row/2 + 1
    O = np.zeros((B, 48, 33, 66), np.float32)
    O[:, :, 1:33, 1:65] = x[:, :48, 1::2, :]            # di=1: i_pad = (x-row+1)/2
    x1hp[:, 64:112] = O
    x1hp[:, 48:64, 1:33, :] = O[:, 0:16, 0:32, :]
    x1hp[:, 112:128, 1:33, :] = O[:, 16:32, 0:32, :]
    x1hp = _bf(x1hp)
    # lone-tap rows for channels 32:48, pre-shifted for the SK tile's gap
    # (engine copies can't start at partition 48, so these come via DMA)
    lone16 = np.zeros((B, 16, 33 * 66), np.float32)
    lone16[:, :, 0:33*66-2] = O[:, 32:48].reshape(B, 16, 33 * 66)[:, :, 2:]
    lone16 = _bf(lone16)

    kin = np.empty((2, 96), np.int64)
    for dj in range(2):
        kin[dj] = [_down_idx(p, dj) for p in range(96)]

    # ---- conv_hi weights: 4 taps x [128, 384] ----
    # tap kj=0,1: merged ki=1&2: rows 0-47 w[.,c,1,kj], rows 64-111 w[.,c,2,kj]
    # tap kj=2:   merged + lone replicas at 48-63 (ch 0:16), 112-127 (ch 16:32)
    # tap 3 (SK): rows 0-47 w[.,c,0,0], 48-63 lone ch 32:48, 64-111 w[.,c,0,1]
    w_hi_eff = w_hi.copy()
    w_hi_eff[192:] *= scale2[:, None, None, None]
    w_hi_r = w_hi_eff[_ROWS]               # [384, 48, 3, 3] in PSUM row order
    lhsT_hi = np.zeros((128, 4, 384), np.float32)
    for kj in range(3):
        lhsT_hi[0:48, kj] = w_hi_r[:, :, 1, kj].T
        lhsT_hi[64:112, kj] = w_hi_r[:, :, 2, kj].T
    lhsT_hi[48:64, 2] = w_hi_r[:, 0:16, 0, 2].T
    lhsT_hi[112:128, 2] = w_hi_r[:, 16:32, 0, 2].T
    lhsT_hi[0:48, 3] = w_hi_r[:, :, 0, 0].T
    lhsT_hi[48:64, 3] = w_hi_r[:, 32:48, 0, 2].T
    lhsT_hi[64:112, 3] = w_hi_r[:, :, 0, 1].T
    lhsT_hi = _bf(lhsT_hi.reshape(128, 4 * 384))

    # ---- conv_lo weights: 15 taps (t*3+ki) x [128, 384] ----
    # y2 dj=0 lives in T1[0:96], dj=1 in T2[0:96] (written there by epi1);
    # replica rows 96:128 and T3 are filled by SBUF->SBUF DMA.
    # t=0: T1[0:96] sigma=0 dj0; t=1: T2[0:96] sigma=0 dj1;
    # t=2: T1[0:128] sigma=1 (96:128 = dj1[0:32] replica);
    # t=3: T2[0:128] sigma=2 (96:128 = dj0[64:96] replica);
    # t=4: T3 (0:64 = dj1[32:96] sigma=1 pre-shifted, 64:128 = dj0[0:64] sigma=2)
    w_lo_eff = w_lo.copy()
    w_lo_eff[192:] *= scale1[:, None, None, None]
    w_lo_eff = w_lo_eff / scale2[None, :, None, None]      # y2 stored pre-scaled
    w_lo_r = w_lo_eff[_ROWS]               # [384, 192, 3, 3]
    lhsT_lo = np.zeros((128, 5, 3, 384), np.float32)
    for ki in range(3):
        lhsT_lo[0:96, 0, ki] = w_lo_r[:, kin[0], ki, 0].T
        lhsT_lo[0:96, 1, ki] = w_lo_r[:, kin[1], ki, 0].T
        lhsT_lo[0:96, 2, ki] = w_lo_r[:, kin[0], ki, 1].T
        lhsT_lo[96:128, 2, ki] = w_lo_r[:, kin[1][0:32], ki, 1].T
        lhsT_lo[0:96, 3, ki] = w_lo_r[:, kin[1], ki, 2].T
        lhsT_lo[96:128, 3, ki] = w_lo_r[:, kin[0][64:96], ki, 2].T
        lhsT_lo[0:64, 4, ki] = w_lo_r[:, kin[1][32:96], ki, 1].T
        lhsT_lo[64:128, 4, ki] = w_lo_r[:, kin[0][0:64], ki, 2].T
    lhsT_lo = _bf(lhsT_lo.reshape(128, 15 * 384))

    # ---- output scatter indices: out row = s*384 + sigma(pre_channel) ----
    sigma = np.zeros(384, np.int64)
    oo, cc = np.nonzero(perm_w)
    sigma[cc] = oo
    scatter_idx = np.zeros((96, 4 * SPS), np.int32)
    for t in range(4):
        dj = t % 2
        base = 0 if t < 2 else 192
        for s in range(SPS):
            scatter_idx[:, t*SPS+s] = s * 384 + sigma[base + kin[dj]]

    # ---- exp biases ln(scale) per stored partition ----
    ebias = np.zeros((96, 4), np.float32)
    for dj in range(2):
        ebias[:, 0+dj] = np.log(scale2[kin[dj]])
        ebias[:, 2+dj] = np.log(scale1[kin[dj]])

    per_core = []
    for ci in range(N_CORES):
        sl = slice(ci * SPS, (ci + 1) * SPS)
        per_core.append(dict(
            x1hp=np.ascontiguousarray(x1hp[sl].reshape(SPS, 128, 33 * 66)),
            lone16=np.ascontiguousarray(lone16[sl]),
            x1h=np.ascontiguousarray(x1h[sl].reshape(SPS, 96, 32 * 64)),
            x2h=np.ascontiguousarray(x2h[sl].reshape(SPS, 96, 32 * 64)),
            lhsT_hi=lhsT_hi, lhsT_lo=lhsT_lo, ebias=ebias, scatter_idx=scatter_idx,
            zb=np.zeros((112, 33 * 66), ml_dtypes.bfloat16),
        ))
    return per_core


# ---------------------------------------------------------------------------
# device kernel
# ---------------------------------------------------------------------------
def build_kernel():
    nc = bass.Bass()
    p_x1hp = nc.declare_dram_parameter("x1hp", [SPS, 128, 33 * 66], BF16, isOutput=False)
    p_l16 = nc.declare_dram_parameter("lone16", [SPS, 16, 33 * 66], BF16, isOutput=False)
    p_x1h = nc.declare_dram_parameter("x1h", [SPS, 96, 32 * 64], BF16, isOutput=False)
    p_x2h = nc.declare_dram_parameter("x2h", [SPS, 96, 32 * 64], BF16, isOutput=False)
    p_whi = nc.declare_dram_parameter("lhsT_hi", [128, 4 * 384], BF16, isOutput=False)
    p_wlo = nc.declare_dram_parameter("lhsT_lo", [128, 15 * 384], BF16, isOutput=False)
    p_eb = nc.declare_dram_parameter("ebias", [96, 4], F32, isOutput=False)
    p_idx = nc.declare_dram_parameter("scatter_idx", [96, 4 * SPS], I32, isOutput=False)
    p_zb = nc.declare_dram_parameter("zb", [112, 33 * 66], BF16, isOutput=False)
    p_out = nc.declare_dram_parameter("out", [SPS * 384, 1024], F32, isOutput=True)

    with TileContext(nc) as tc:
        with (
            tc.tile_pool(name="wt", bufs=1) as wt,
            tc.tile_pool(name="xin", bufs=2) as xin,
            tc.tile_pool(name="sk", bufs=2) as skp,
            tc.tile_pool(name="ttile", bufs=2) as tpool,
            tc.tile_pool(name="etile", bufs=2) as epool,
            tc.tile_pool(name="tmp", bufs=3) as mpool,
            tc.tile_pool(name="ytile", bufs=1) as ypool,
            tc.tile_pool(name="yc", bufs=2) as ycpool,
            tc.tile_pool(name="ps", bufs=8, space="PSUM") as ps,
        ):
            whi = wt.tile([128, 4 * 384], BF16)
            wlo = wt.tile([128, 15 * 384], BF16)
            eb = wt.tile([96, 4], F32)
            tix = wt.tile([96, 4 * SPS], I32)
            nc.sync.dma_start(out=whi, in_=p_whi[:])

            sk_t = [ypool.tile([128, 33 * 66], BF16, tag=f"sk_{sl}", name=f"sk_{sl}")
                    for sl in range(2)]
            # T1/T2 rows 0:96 are y2 (dj=0/1), written directly by epi1;
            # rows 96:128 and all of T3 are DMA-filled replicas for the
            # sigma=1,2 conv_lo taps.
            stk = [[ypool.tile([128, 34 * 34], BF16, tag=f"T_{sl}_{k}", name=f"T_{sl}_{k}")
                    for k in range(3)] for sl in range(2)]

            state = {}

            def load_dma(s, defer_u=False):
                x1hp = xin.tile([128, 33 * 66], BF16, tag="x1hp", name=f"x1hp_{s}")
                x1h = xin.tile([96, 32 * 64], BF16, tag="x1h", name=f"x1h_{s}")
                x2h = xin.tile([96, 32 * 64], BF16, tag="x2h", name=f"x2h_{s}")
                sk = sk_t[s % 2]
                if defer_u:
                    # sample 0: rows 0:18 first so conv_hi n0 can start sooner
                    nc.sync.dma_start(out=x1hp[:, 0:18*66+2], in_=p_x1hp[s][:, 0:18*66+2])
                    nc.sync.dma_start(out=sk[48:64, :], in_=p_l16[s])
                    nc.sync.dma_start(out=x1hp[:, 18*66+2:], in_=p_x1hp[s][:, 18*66+2:])
                else:
                    nc.sync.dma_start(out=x1hp, in_=p_x1hp[s])
                    nc.sync.dma_start(out=sk[48:64, :], in_=p_l16[s])
                    nc.sync.dma_start(out=x1h, in_=p_x1h[s])
                    nc.sync.dma_start(out=x2h, in_=p_x2h[s])
                state[s] = dict(
                    x1hp_f=x1hp, sk_f=sk, x1h_f=x1h, x2h_f=x2h,
                    x1hp=x1hp.rearrange("p (i w) -> p i w", i=33),
                    sk=sk.rearrange("p (i w) -> p i w", i=33),
                    x1h=x1h.rearrange("p (i w) -> p i w", i=32),
                    x2h=x2h.rearrange("p (i w) -> p i w", i=32),
                    y2v=[stk[s % 2][dj][0:96, :].rearrange("p (i w) -> p i w", i=34)
                         for dj in range(2)],
                    stkv=[stk[s % 2][k].rearrange("p (i w) -> p i w", i=34)
                          for k in range(3)],
                )

            def sk_copy(s, split=False):
                # SK: ki=0 taps for kj=0 (rows 0-47) and kj=1 (rows 64-111,
                # data shifted one column left); rows 48-63 hold the lone-tap
                # replicas for channels 32:48 (DMA'd from host).
                x1hp, sk = state[s]["x1hp_f"], state[s]["sk_f"]
                if split:   # sample 0: n0 rows first, matching the split load
                    c = 18 * 66
                    nc.vector.tensor_copy(sk[0:48, 0:c], x1hp[64:112, 0:c])
                    nc.vector.tensor_copy(sk[64:112, 0:c], x1hp[64:112, 1:c+1])
                    nc.vector.tensor_copy(sk[0:48, c:], x1hp[64:112, c:])
                    nc.vector.tensor_copy(sk[64:112, c:33*66-1], x1hp[64:112, c+1:33*66])
                else:
                    nc.vector.tensor_copy(sk[0:48, :], x1hp[64:112, :])
                    nc.vector.tensor_copy(sk[64:112, 0:33*66-1], x1hp[64:112, 1:33*66])

            def conv_hi(s):
                st = state[s]
                a1 = [[ps.tile([128, NCHUNK], F32, tag="psum", name=f"a1_{s}_{m}_{n}")
                       for n in range(2)] for m in range(3)]
                xv, skv = st["x1hp"], st["sk"]
                for n in range(2):
                    for m in range(3):
                        a = a1[m][n]
                        nc.tensor.matmul(
                            a[:, :], whi[0:112, 128*m:128*m + 128],
                            xv[0:112, 16*n+1:16*n+17, 0:63:2], start=True, stop=False)
                        nc.tensor.matmul(
                            a[:, :], whi[0:112, 384 + 128*m:384 + 128*m + 128],
                            xv[0:112, 16*n+1:16*n+17, 1:64:2], start=False, stop=False)
                        nc.tensor.matmul(
                            a[:, :], whi[0:128, 2*384 + 128*m:2*384 + 128*m + 128],
                            xv[0:128, 16*n+1:16*n+17, 2:65:2], start=False, stop=False)
                        nc.tensor.matmul(
                            a[:, :], whi[0:112, 3*384 + 128*m:3*384 + 128*m + 128],
                            skv[0:112, 16*n:16*n+16, 0:63:2], start=False, stop=True)
                st["a1"] = a1

            def epi1(s):
                st = state[s]
                self_affine(nc, tpool, epool, mpool, st["a1"], eb, 0, st["x2h"],
                            st["y2v"], y2_mode=True)
                # contiguous copies of y2 interior for the output scatter,
                # then scatter y2's channels now (off the critical path).
                y2c = [ycpool.tile([96, 1024], BF16, tag=f"y2c_{dj}", name=f"y2c_{s}_{dj}")
                       for dj in range(2)]
                for dj in range(2):
                    nc.vector.tensor_copy(
                        y2c[dj].rearrange("p (i w) -> p i w", i=32),
                        st["y2v"][dj][0:96, 1:33, 1:33])
                    nc.gpsimd.indirect_dma_start(
                        out=p_out[:],
                        out_offset=bass.IndirectOffsetOnAxis(
                            ap=tix[:, (2+dj)*SPS+s:(2+dj)*SPS+s+1], axis=0),
                        in_=y2c[dj][:, :], in_offset=None)

            def build_T(s):
                # Replica fills via SBUF->SBUF DMA (engine copies crawl under
                # SBUF port contention with the matmul stream).  T3 rows 0:64
                # (sigma=1) are stored shifted one column right so a single
                # col-base-2 AP works.
                T1, T2, T3 = stk[s % 2]
                nc.sync.dma_start(out=T1[96:128, :], in_=T2[0:32, :])
                nc.sync.dma_start(out=T2[96:128, :], in_=T1[64:96, :])
                nc.sync.dma_start(out=T3[0:64, 1:34*34], in_=T2[32:96, 0:34*34-1])
                nc.sync.dma_start(out=T3[64:128, :], in_=T1[0:64, :])

            def conv_lo(s):
                st = state[s]
                a2 = [[ps.tile([128, NCHUNK], F32, tag="psum", name=f"a2_{s}_{m}_{n}")
                       for n in range(2)] for m in range(3)]
                stkv = st["stkv"]
                # (view, K, col base); taps t*3+ki match lhsT_lo layout
                tiles = [(stkv[0], 96, 0), (stkv[1], 96, 0),
                         (stkv[0], 128, 1), (stkv[1], 128, 2), (stkv[2], 128, 2)]
                for n in range(2):
                    for m in range(3):
                        idx = 0
                        for t, (view, K, cb) in enumerate(tiles):
                            for ki in range(3):
                                rhs = view[0:K, 16*n+ki:16*n+ki+16, cb:cb+32]
                                nc.tensor.matmul(
                                    a2[m][n][:, :],
                                    wlo[0:K, (t*3+ki)*384 + 128*m:(t*3+ki)*384 + 128*m + 128],
                                    rhs, start=(idx == 0), stop=(idx == 14))
                                idx += 1
                st["a2"] = a2

            def epi2(s):
                st = state[s]
                y1t = [mpool.tile([96, 1024], BF16, tag=f"y1_{dj}", name=f"y1_{s}_{dj}")
                       for dj in range(2)]
                y1v = [y1t[dj].rearrange("p (n f) -> p n f", n=2) for dj in range(2)]

                def scat(t):
                    nc.gpsimd.indirect_dma_start(
                        out=p_out[:],
                        out_offset=bass.IndirectOffsetOnAxis(
                            ap=tix[:, t*SPS+s:t*SPS+s+1], axis=0),
                        in_=y1t[t][:, :], in_offset=None)

                self_affine(nc, tpool, epool, mpool, st["a2"], eb, 2, st["x1h"],
                            y1v, y2_mode=False, scatter_cb=scat)
                st["y1t"] = y1t

            load_dma(0, defer_u=True); sk_copy(0, split=True)
            conv_hi(0)
            # y2 borders in T1/T2 rows 0:96 zeroed (epi1 writes interior only)
            for sl in range(2):
                for k in range(2):
                    nc.sync.dma_start(out=stk[sl][k][0:96, :], in_=p_zb[0:96, 0:34*34])
            nc.sync.dma_start(out=eb, in_=p_eb[:])
            nc.sync.dma_start(out=tix, in_=p_idx[:])
            nc.sync.dma_start(out=state[0]["x1h_f"], in_=p_x1h[0])
            nc.sync.dma_start(out=state[0]["x2h_f"], in_=p_x2h[0])
            load_dma(1)
            # wlo split: sigma=0 taps (first 6 matmuls of each chunk) first,
            # the rest lands behind the build_T(0) replica fills
            nc.sync.dma_start(out=wlo[:, 0:6*384], in_=p_wlo[:, 0:6*384])
            epi1(0); build_T(0)
            nc.sync.dma_start(out=wlo[:, 6*384:], in_=p_wlo[:, 6*384:])
            for s in range(SPS):
                if s + 1 < SPS:
                    if s >= 1:
                        load_dma(s + 1)
                    sk_copy(s + 1)
                    conv_hi(s + 1); epi1(s + 1); build_T(s + 1)
                conv_lo(s); epi2(s)
    return nc


def self_affine(nc, tpool, epool, mpool, a, eb, eb_base, xh_v, yv, y2_mode,
                scatter_cb=None):
    """y[dj] = u[dj] * exp(2*tanh(0.2*a_s) + ln_scale) + a_t.

    a: 3x2 PSUM tiles [128, 512] with rows ordered
       [E0(96), T0(0:32) | T0(32:96), E1(0:64) | T1(96), E1(64:96)].

    y2_mode: output rows are the padded y2 interior, and the n-chunks are
    split asymmetrically: n0 ops produce output rows oh 0-15; a small
    fragment then produces row oh=16 (the first row of the n1 PSUM chunk) so
    conv_lo's n0 half can start before the n1 bulk (rows 17-31) is done."""
    t = [tpool.tile([96, 1024], F32, tag=f"t{eb_base}_{dj}", name=f"t{eb_base}_{dj}") for dj in range(2)]
    e = [epool.tile([96, 1024], BF16, tag=f"e{eb_base}_{dj}", name=f"e{eb_base}_{dj}") for dj in range(2)]
    rr = lambda ap, i=16: ap.rearrange("p (i w) -> p i w", i=i)

    def tanh_exp(n, cs=slice(0, 512)):
        # dj=0 needs only the m0 chunk: emit its exp before the m1/m2 tanhs
        # so the dj=0 chain never queues behind later matmul chunks
        ns = slice(n * 512 + cs.start, n * 512 + cs.stop)
        ebs = lambda dj: eb[0:96, eb_base + dj:eb_base + dj + 1]
        nc.scalar.activation(t[0][0:96, ns], a[0][n][0:96, cs], AF.Tanh, scale=0.2)
        nc.scalar.activation(e[0][0:96, ns], t[0][0:96, ns], AF.Exp, bias=ebs(0), scale=2.0)
        nc.scalar.activation(t[1][0:64, ns], a[1][n][64:128, cs], AF.Tanh, scale=0.2)
        nc.scalar.activation(t[1][64:96, ns], a[2][n][96:128, cs], AF.Tanh, scale=0.2)
        nc.scalar.activation(e[1][0:96, ns], t[1][0:96, ns], AF.Exp, bias=ebs(1), scale=2.0)

    def mul_add(dj, n, dst, i0, i1, csl):
        # tmp = u * E over output rows oh in [16n+i0, 16n+i1)
        nr = i1 - i0
        tmp = mpool.tile([96, 512], BF16, tag=f"tmp{eb_base}_{dj}", name=f"tmp{eb_base}_{dj}")
        uview = xh_v[0:96, 16*n+i0:16*n+i1, dj:dj+63:2]
        ev = rr(e[dj][0:96, n*512 + i0*32:n*512 + i1*32], nr)
        tmv = rr(tmp[0:96, 0:nr*32], nr)
        nc.gpsimd.tensor_tensor(out=tmv[0:96], in0=uview, in1=ev, op=MUL)
        tv = rr(tmp[0:96, 0:nr*32], nr)
        pr = lambda ap: rr(ap[:, csl], nr)
        if dj == 0:
            nc.vector.tensor_tensor(out=dst[0:32], in0=tv[0:32], in1=pr(a[0][n][96:128, :]), op=ADD)
            nc.vector.tensor_tensor(out=dst[32:64], in0=tv[32:64], in1=pr(a[1][n][0:32, :]), op=ADD)
            nc.vector.tensor_tensor(out=dst[64:96], in0=tv[64:96], in1=pr(a[1][n][32:64, :]), op=ADD)
        else:
            nc.vector.tensor_tensor(out=dst[0:96], in0=tv[0:96], in1=pr(a[2][n][0:96, :]), op=ADD)

    # ---- n = 0: full 16 rows ----
    tanh_exp(0)
    for dj in range(2):
        if y2_mode:
            dst = yv[dj][0:96, 1:17, 1:33]
        else:
            dst = rr(yv[dj][0:96, 0, :])
        mul_add(dj, 0, dst, 0, 16, slice(0, 512))

    if not y2_mode:
        tanh_exp(1)
        for dj in range(2):
            mul_add(dj, 1, rr(yv[dj][0:96, 1, :]), 0, 16, slice(0, 512))
        # scat(0) only waits on the dj=0 chain (complete before the last
        # matmul chunk); its issue overlaps the dj=1 adds
        if scatter_cb is not None:
            scatter_cb(0); scatter_cb(1)
        return

    # ---- fragment: output row oh=16 (first col-block of the n1 chunks) ----
    fsl = slice(512, 544)
    nc.scalar.activation(t[0][0:96, fsl], a[0][1][0:96, 0:32], AF.Tanh, scale=0.2)
    nc.scalar.activation(t[1][0:64, fsl], a[1][1][64:128, 0:32], AF.Tanh, scale=0.2)
    nc.scalar.activation(t[1][64:96, fsl], a[2][1][96:128, 0:32], AF.Tanh, scale=0.2)
    for dj in range(2):
        nc.scalar.activation(e[dj][0:96, fsl], t[dj][0:96, fsl], AF.Exp,
                             bias=eb[0:96, eb_base + dj:eb_base + dj + 1], scale=2.0)
        ftmp = mpool.tile([96, 32], BF16, tag=f"ftmp_{dj}", name=f"ftmp_{dj}")
        uview = xh_v[0:96, 16:17, dj:dj+63:2]
        nc.gpsimd.tensor_tensor(out=ftmp[0:96, :].rearrange("p (i w) -> p i w", i=1),
                                in0=uview,
                                in1=e[dj][0:96, fsl].rearrange("p (i w) -> p i w", i=1), op=MUL)
        dst = yv[dj][0:96, 17:18, 1:33]
        fv = ftmp[0:96, :].rearrange("p (i w) -> p i w", i=1)
        pr = lambda ap: ap[:, 0:32].rearrange("p (i w) -> p i w", i=1)
        if dj == 0:
            nc.vector.tensor_tensor(out=dst[0:32], in0=fv[0:32], in1=pr(a[0][1][96:128, :]), op=ADD)
            nc.vector.tensor_tensor(out=dst[32:64], in0=fv[32:64], in1=pr(a[1][1][0:32, :]), op=ADD)
            nc.vector.tensor_tensor(out=dst[64:96], in0=fv[64:96], in1=pr(a[1][1][32:64, :]), op=ADD)
        else:
            nc.vector.tensor_tensor(out=dst[0:96], in0=fv[0:96], in1=pr(a[2][1][0:96, :]), op=ADD)

    # ---- n = 1 bulk: rows oh 17-31 (output rows 18-33) ----
    tanh_exp(1)
    for dj in range(2):
        dst = yv[dj][0:96, 18:33, 1:33]
        mul_add(dj, 1, dst, 1, 16, slice(32, 512))


# ---------------------------------------------------------------------------
# entry point
# ---------------------------------------------------------------------------
_CACHE = {}

def _get_nc():
    if "nc" not in _CACHE:
        nc = build_kernel()
        _split_excess_waits(nc)
        _CACHE["nc"] = nc
    return _CACHE["nc"]


def _split_excess_waits(nc):
    """This walrus allows 1 sync wait per instruction (2 for EventSemaphore);
    hoist overflow waits into standalone EventSemaphore insts."""
    import bass_rust
    n_fix = 0
    for fn in nc.m.functions:
        for blk in fn.blocks:
            insts = blk.instructions
            out = []
            for inst in insts:
                si = inst.sync_info
                cap = 2 if inst.opcode == "EventSemaphore" else 1
                if si is not None and len(si.on_wait) > cap:
                    waits = list(si.on_wait)
                    keep, extra = waits[:cap], waits[cap:]
                    for ci in range(0, len(extra), 2):
                        n_fix += 1
                        ev = mybir.InstEventSemaphore(name=f"I-waitfix-{n_fix}", ins=[], outs=[])
                        ev.engine = inst.engine
                        ev.sync_info = bass_rust.SyncInfo(on_wait=extra[ci:ci+2], on_update=[])
                        nc.register_instruction(ev, overwrite=True)
                        out.append(ev)
                    inst.sync_info = bass_rust.SyncInfo(on_wait=keep, on_update=list(si.on_update))
                out.append(inst)
            if len(out) != len(insts):
                blk.instructions = out
    return n_fix


def _install_profile_shim():
    import types
    name = "antenv.axon_hooks"
    if name in sys.modules:
        return
    try:
        from trn_agent_boot.trn_boot import _ntff_profile_via_ctypes
        hook = _ntff_profile_via_ctypes('/opt/axon/libaxon_pjrt.so')
    except Exception:
        hook = None
    mod = types.ModuleType(name)
    mod._hook = hook
    mod.get_axon_ntff_profile_hook = lambda: mod._hook
    mod.set_axon_ntff_profile_hook = lambda h: setattr(mod, '_hook', h)
    sys.modules[name] = mod


def run(inputs, trace=False):
    _install_profile_shim()
    per_core = _prepare(**inputs)
    nc = _get_nc()
    res = run_bass_kernel_spmd(nc, per_core, core_ids=list(range(N_CORES)), trace=trace)
    outs = [res.results[i]["out"].reshape(SPS, 384, 32, 32) for i in range(N_CORES)]
    full = np.concatenate(outs, axis=0).astype(np.float32)
    return full, res


def kernel(**inputs):
    full, _ = run(inputs, trace=False)
    return full


# revision 34
# speedup vs baseline: 1.0631x; 1.0631x over previous
"""Trainium2 Bass kernel for nn_AIO_DownsampleCouplingBlock.

Reference computation (B=32, C=96, H=W=64, split 48/48):
  x1, x2 = x[:, :48], x[:, 48:]
  y2 = down(x2);  a1 = conv3x3_s2(x1, w_hi) + b_hi
  y2 = y2 * exp(2*tanh(0.2*a1[:192])) + a1[192:]
  y1 = down(x1);  a2 = conv3x3_s1(y2, w_lo) + b_lo
  y1 = y1 * exp(2*tanh(0.2*a2[:192])) + a2[192:]
  out = perm_w @ (concat(y1, y2) * scale + offset)   (channel matmul)
  scale = 0.2*softplus(0.5*act_norm), offset = act_offset

Data-parallel over batch: 4 samples per core on 8 cores.  Everything static
(actnorm folding, channel reorders, permutation indices) is precomputed on
host in numpy.  On device:
  - conv_hi: 4 bf16 matmuls per output chunk.  x1hp [128, 33, 66] holds even
    half-rows at partitions 0-47, odd at 64-111; the 48 "lone" rows
    (ki=0,kj=2) are stuffed into the gaps: partitions 48-63 / 112-127 hold
    row-shifted odd-plane replicas for 32 channels (weights live only in the
    kj=2 tap, which reads K=128), and the SK tile's gap 48-63 holds the last
    16 (built by one extra on-device copy).  Vertical taps ki=1,2 merge into
    one matmul per kj; ki=0 kj=0,1 merge via the column-shifted SK copy.
  - conv_lo: 15 matmuls per output chunk (vs 18 naive).  sigma=0 taps read
    the two 96-partition y2 tiles directly; sigma=1,2 read three dense
    K=128 tiles T1/T2/T3 built per sample by contiguous engine copies of y2
    (T3 mixes sigma by storing its sigma=1 rows pre-shifted one column).
  - conv output rows are reordered (host) so every PSUM->SBUF eviction piece
    starts at partition 0/32/64/96 (hardware window rule):
      [E0(96), T0(0:32) | T0(32:96), E1(0:64) | T1(96), E1(64:96)]
  - epilogue: tanh on ScalarE (scale=0.2), exp on ScalarE with ln(actnorm
    scale) folded into the bias, mul on VectorE, add on VectorE reading the
    conv t-half directly from PSUM.
  - actnorm scale is folded into: exp bias (for the multiplicative half),
    conv t-half weight rows, and 1/scale into conv_lo's input-channel
    weights (y2 is stored pre-scaled).
  - the channel permutation + output writes are indirect-DMA scatters of the
    (bf16) y tiles straight to the f32 DRAM output, one 96-row scatter per
    (tile, sample).
  - the tensor queue is software-pipelined: conv_hi(s+1) + epi1(s+1) are
    emitted before conv_lo(s), so the epi1 y2 chain always has a full
    conv_lo of cover and the PE never waits on the epilogue.
"""
import sys, os
sys.path.insert(0, '/opt/trn_rl_repo')
import numpy as np
import ml_dtypes

import concourse.bass as bass
import concourse.mybir as mybir
from concourse.tile import TileContext
from concourse.bass_utils import run_bass_kernel_spmd

F32 = mybir.dt.float32
BF16 = mybir.dt.bfloat16
I32 = mybir.dt.int32
AF = mybir.ActivationFunctionType
MUL = mybir.AluOpType.mult
ADD = mybir.AluOpType.add

N_CORES = 8
B = 32
SPS = B // N_CORES            # samples per core
NCHUNK = 512                  # matmul free size (16 rows x 32 cols)

# stored partition p (0..95) of a (dj) tile -> down-channel k = 4c + 2di + dj
def _down_idx(p, dj):
    di, c = divmod(p, 48)
    return 4 * c + 2 * di + dj

def _psum_rows():
    e0 = [_down_idx(p, 0) for p in range(96)]
    e1 = [_down_idx(p, 1) for p in range(96)]
    t0 = [192 + _down_idx(p, 0) for p in range(96)]
    t1 = [192 + _down_idx(p, 1) for p in range(96)]
    return np.array(e0 + t0[:32] + t0[32:] + e1[:64] + t1 + e1[64:], np.int64)

_ROWS = _psum_rows()

def _bf(a):
    return np.ascontiguousarray(a).astype(ml_dtypes.bfloat16)


# ---------------------------------------------------------------------------
# host-side preprocessing
# ---------------------------------------------------------------------------
def _prepare(x, w_hi, b_hi, w_lo, b_lo, act_norm, act_offset, perm_w):
    x = np.asarray(x, np.float32)
    w_hi = np.asarray(w_hi, np.float32); w_lo = np.asarray(w_lo, np.float32)
    b_hi = np.asarray(b_hi, np.float32); b_lo = np.asarray(b_lo, np.float32)
    act_norm = np.asarray(act_norm, np.float32).reshape(-1)
    act_offset = np.asarray(act_offset, np.float32).reshape(-1)
    perm_w = np.asarray(perm_w, np.float32)

    scale = 0.2 * np.log1p(np.exp(0.5 * act_norm))          # softplus, beta=0.5
    assert np.allclose(b_hi, 0) and np.allclose(b_lo, 0) and \
        np.allclose(act_offset, 0), "nonzero conv bias / actnorm offset not implemented"
    scale1, scale2 = scale[:192], scale[192:]

    # ---- input layouts (bf16) ----
    def halfrows(xc):                      # xc [B, 48, 64, 64] -> [B, 96, 32, 64]
        v = xc.reshape(B, 48, 32, 2, 64).transpose(0, 3, 1, 2, 4)
        return v.reshape(B, 96, 32, 64)
    x1h = _bf(halfrows(x[:, :48]))
    x2h = _bf(halfrows(x[:, 48:]))
    # x1hp [B, 128, 33, 66]: 0:48 even half-rows, 64:112 odd half-rows,
    # 48:64 / 112:128 = odd planes shifted down one row (lone-tap replicas,
    # channels 0:16 / 16:32), read only by the K=128 kj=2 tap.
    x1hp = np.zeros((B, 128, 33, 66), np.float32)
    x1hp[:, 0:48, 1:33, 1:65] = x[:, :48, 0::2, :]      # di=0: i_pad = x-row/2 + 1
    O = np.zeros((B, 48, 33, 66), np.float32)
    O[:, :, 1:33, 1:65] = x[:, :48, 1::2, :]            # di=1: i_pad = (x-row+1)/2
    x1hp[:, 64:112] = O
    x1hp[:, 48:64, 1:33, :] = O[:, 0:16, 0:32, :]
    x1hp[:, 112:128, 1:33, :] = O[:, 16:32, 0:32, :]
    x1hp = _bf(x1hp)
    # lone-tap rows for channels 32:48, pre-shifted for the SK tile's gap
    # (engine copies can't start at partition 48, so these come via DMA)
    lone16 = np.zeros((B, 16, 33 * 66), np.float32)
    lone16[:, :, 0:33*66-2] = O[:, 32:48].reshape(B, 16, 33 * 66)[:, :, 2:]
    lone16 = _bf(lone16)

    kin = np.empty((2, 96), np.int64)
    for dj in range(2):
        kin[dj] = [_down_idx(p, dj) for p in range(96)]

    # ---- conv_hi weights: 4 taps x [128, 384] ----
    # tap kj=0,1: merged ki=1&2: rows 0-47 w[.,c,1,kj], rows 64-111 w[.,c,2,kj]
    # tap kj=2:   merged + lone replicas at 48-63 (ch 0:16), 112-127 (ch 16:32)
    # tap 3 (SK): rows 0-47 w[.,c,0,0], 48-63 lone ch 32:48, 64-111 w[.,c,0,1]
    w_hi_eff = w_hi.copy()
    w_hi_eff[192:] *= scale2[:, None, None, None]
    w_hi_r = w_hi_eff[_ROWS]               # [384, 48, 3, 3] in PSUM row order
    lhsT_hi = np.zeros((128, 4, 384), np.float32)
    for kj in range(3):
        lhsT_hi[0:48, kj] = w_hi_r[:, :, 1, kj].T
        lhsT_hi[64:112, kj] = w_hi_r[:, :, 2, kj].T
    lhsT_hi[48:64, 2] = w_hi_r[:, 0:16, 0, 2].T
    lhsT_hi[112:128, 2] = w_hi_r[:, 16:32, 0, 2].T
    lhsT_hi[0:48, 3] = w_hi_r[:, :, 0, 0].T
    lhsT_hi[48:64, 3] = w_hi_r[:, 32:48, 0, 2].T
    lhsT_hi[64:112, 3] = w_hi_r[:, :, 0, 1].T
    lhsT_hi = _bf(lhsT_hi.reshape(128, 4 * 384))

    # ---- conv_lo weights: 15 taps (t*3+ki) x [128, 384] ----
    # y2 dj=0 lives in T1[0:96], dj=1 in T2[0:96] (written there by epi1);
    # replica rows 96:128 and T3 are filled by SBUF->SBUF DMA.
    # t=0: T1[0:96] sigma=0 dj0; t=1: T2[0:96] sigma=0 dj1;
    # t=2: T1[0:128] sigma=1 (96:128 = dj1[0:32] replica);
    # t=3: T2[0:128] sigma=2 (96:128 = dj0[64:96] replica);
    # t=4: T3 (0:64 = dj1[32:96] sigma=1 pre-shifted, 64:128 = dj0[0:64] sigma=2)
    w_lo_eff = w_lo.copy()
    w_lo_eff[192:] *= scale1[:, None, None, None]
    w_lo_eff = w_lo_eff / scale2[None, :, None, None]      # y2 stored pre-scaled
    w_lo_r = w_lo_eff[_ROWS]               # [384, 192, 3, 3]
    lhsT_lo = np.zeros((128, 5, 3, 384), np.float32)
    for ki in range(3):
        lhsT_lo[0:96, 0, ki] = w_lo_r[:, kin[0], ki, 0].T
        lhsT_lo[0:96, 1, ki] = w_lo_r[:, kin[1], ki, 0].T
        lhsT_lo[0:96, 2, ki] = w_lo_r[:, kin[0], ki, 1].T
        lhsT_lo[96:128, 2, ki] = w_lo_r[:, kin[1][0:32], ki, 1].T
        lhsT_lo[0:96, 3, ki] = w_lo_r[:, kin[1], ki, 2].T
        lhsT_lo[96:128, 3, ki] = w_lo_r[:, kin[0][64:96], ki, 2].T
        lhsT_lo[0:64, 4, ki] = w_lo_r[:, kin[1][32:96], ki, 1].T
        lhsT_lo[64:128, 4, ki] = w_lo_r[:, kin[0][0:64], ki, 2].T
    lhsT_lo = _bf(lhsT_lo.reshape(128, 15 * 384))

    # ---- output scatter indices: out row = s*384 + sigma(pre_channel) ----
    sigma = np.zeros(384, np.int64)
    oo, cc = np.nonzero(perm_w)
    sigma[cc] = oo
    scatter_idx = np.zeros((96, 4 * SPS), np.int32)
    for t in range(4):
        dj = t % 2
        base = 0 if t < 2 else 192
        for s in range(SPS):
            scatter_idx[:, t*SPS+s] = s * 384 + sigma[base + kin[dj]]

    # ---- exp biases ln(scale) per stored partition ----
    ebias = np.zeros((96, 4), np.float32)
    for dj in range(2):
        ebias[:, 0+dj] = np.log(scale2[kin[dj]])
        ebias[:, 2+dj] = np.log(scale1[kin[dj]])

    per_core = []
    for ci in range(N_CORES):
        sl = slice(ci * SPS, (ci + 1) * SPS)
        per_core.append(dict(
            x1hp=np.ascontiguousarray(x1hp[sl].reshape(SPS, 128, 33 * 66)),
            lone16=np.ascontiguousarray(lone16[sl]),
            x1h=np.ascontiguousarray(x1h[sl].reshape(SPS, 96, 32 * 64)),
            x2h=np.ascontiguousarray(x2h[sl].reshape(SPS, 96, 32 * 64)),
            lhsT_hi=lhsT_hi, lhsT_lo=lhsT_lo, ebias=ebias, scatter_idx=scatter_idx,
            zb=np.zeros((112, 33 * 66), ml_dtypes.bfloat16),
        ))
    return per_core


# ---------------------------------------------------------------------------
# device kernel
# ---------------------------------------------------------------------------
def build_kernel():
    nc = bass.Bass()
    p_x1hp = nc.declare_dram_parameter("x1hp", [SPS, 128, 33 * 66], BF16, isOutput=False)
    p_l16 = nc.declare_dram_parameter("lone16", [SPS, 16, 33 * 66], BF16, isOutput=False)
    p_x1h = nc.declare_dram_parameter("x1h", [SPS, 96, 32 * 64], BF16, isOutput=False)
    p_x2h = nc.declare_dram_parameter("x2h", [SPS, 96, 32 * 64], BF16, isOutput=False)
    p_whi = nc.declare_dram_parameter("lhsT_hi", [128, 4 * 384], BF16, isOutput=False)
    p_wlo = nc.declare_dram_parameter("lhsT_lo", [128, 15 * 384], BF16, isOutput=False)
    p_eb = nc.declare_dram_parameter("ebias", [96, 4], F32, isOutput=False)
    p_idx = nc.declare_dram_parameter("scatter_idx", [96, 4 * SPS], I32, isOutput=False)
    p_zb = nc.declare_dram_parameter("zb", [112, 33 * 66], BF16, isOutput=False)
    p_out = nc.declare_dram_parameter("out", [SPS * 384, 1024], F32, isOutput=True)

    with TileContext(nc) as tc:
        with (
            tc.tile_pool(name="wt", bufs=1) as wt,
            tc.tile_pool(name="xin", bufs=2) as xin,
            tc.tile_pool(name="sk", bufs=2) as skp,
            tc.tile_pool(name="ttile", bufs=2) as tpool,
            tc.tile_pool(name="etile", bufs=2) as epool,
            tc.tile_pool(name="tmp", bufs=3) as mpool,
            tc.tile_pool(name="ytile", bufs=1) as ypool,
            tc.tile_pool(name="yc", bufs=2) as ycpool,
            tc.tile_pool(name="ps", bufs=8, space="PSUM") as ps,
        ):
            whi = wt.tile([128, 4 * 384], BF16)
            wlo = wt.tile([128, 15 * 384], BF16)
            eb = wt.tile([96, 4], F32)
            tix = wt.tile([96, 4 * SPS], I32)
            nc.sync.dma_start(out=whi, in_=p_whi[:])

            sk_t = [ypool.tile([128, 33 * 66], BF16, tag=f"sk_{sl}", name=f"sk_{sl}")
                    for sl in range(2)]
            # T1/T2 rows 0:96 are y2 (dj=0/1), written directly by epi1;
            # rows 96:128 and all of T3 are DMA-filled replicas for the
            # sigma=1,2 conv_lo taps.
            stk = [[ypool.tile([128, 34 * 34], BF16, tag=f"T_{sl}_{k}", name=f"T_{sl}_{k}")
                    for k in range(3)] for sl in range(2)]

            state = {}

            def load_dma(s, defer_u=False):
                x1hp = xin.tile([128, 33 * 66], BF16, tag="x1hp", name=f"x1hp_{s}")
                x1h = xin.tile([96, 32 * 64], BF16, tag="x1h", name=f"x1h_{s}")
                x2h = xin.tile([96, 32 * 64], BF16, tag="x2h", name=f"x2h_{s}")
                sk = sk_t[s % 2]
                if defer_u:
                    # sample 0: rows 0:18 first so conv_hi n0 can start sooner
                    nc.sync.dma_start(out=x1hp[:, 0:18*66+2], in_=p_x1hp[s][:, 0:18*66+2])
                    nc.sync.dma_start(out=sk[48:64, :], in_=p_l16[s])
                    nc.sync.dma_start(out=x1hp[:, 18*66+2:], in_=p_x1hp[s][:, 18*66+2:])
                else:
                    nc.sync.dma_start(out=x1hp, in_=p_x1hp[s])
                    nc.sync.dma_start(out=sk[48:64, :], in_=p_l16[s])
                    nc.sync.dma_start(out=x1h, in_=p_x1h[s])
                    nc.sync.dma_start(out=x2h, in_=p_x2h[s])
                state[s] = dict(
                    x1hp_f=x1hp, sk_f=sk, x1h_f=x1h, x2h_f=x2h,
                    x1hp=x1hp.rearrange("p (i w) -> p i w", i=33),
                    sk=sk.rearrange("p (i w) -> p i w", i=33),
                    x1h=x1h.rearrange("p (i w) -> p i w", i=32),
                    x2h=x2h.rearrange("p (i w) -> p i w", i=32),
                    y2v=[stk[s % 2][dj][0:96, :].rearrange("p (i w) -> p i w", i=34)
                         for dj in range(2)],
                    stkv=[stk[s % 2][k].rearrange("p (i w) -> p i w", i=34)
                          for k in range(3)],
                )

            def sk_copy(s, split=False):
                # SK: ki=0 taps for kj=0 (rows 0-47) and kj=1 (rows 64-111,
                # data shifted one column left); rows 48-63 hold the lone-tap
                # replicas for channels 32:48 (DMA'd from host).
                x1hp, sk = state[s]["x1hp_f"], state[s]["sk_f"]
                if split:   # sample 0: n0 rows first, matching the split load
                    c = 18 * 66
                    nc.vector.tensor_copy(sk[0:48, 0:c], x1hp[64:112, 0:c])
                    nc.vector.tensor_copy(sk[64:112, 0:c], x1hp[64:112, 1:c+1])
                    nc.vector.tensor_copy(sk[0:48, c:], x1hp[64:112, c:])
                    nc.vector.tensor_copy(sk[64:112, c:33*66-1], x1hp[64:112, c+1:33*66])
                else:
                    nc.vector.tensor_copy(sk[0:48, :], x1hp[64:112, :])
                    nc.vector.tensor_copy(sk[64:112, 0:33*66-1], x1hp[64:112, 1:33*66])

            def conv_hi(s):
                st = state[s]
                a1 = [[ps.tile([128, NCHUNK], F32, tag="psum", name=f"a1_{s}_{m}_{n}")
                       for n in range(2)] for m in range(3)]
                xv, skv = st["x1hp"], st["sk"]
                for n in range(2):
                    for m in range(3):
                        a = a1[m][n]
                        nc.tensor.matmul(
                            a[:, :], whi[0:112, 128*m:128*m + 128],
                            xv[0:112, 16*n+1:16*n+17, 0:63:2], start=True, stop=False)
                        nc.tensor.matmul(
                            a[:, :], whi[0:112, 384 + 128*m:384 + 128*m + 128],
                            xv[0:112, 16*n+1:16*n+17, 1:64:2], start=False, stop=False)
                        nc.tensor.matmul(
                            a[:, :], whi[0:128, 2*384 + 128*m:2*384 + 128*m + 128],
                            xv[0:128, 16*n+1:16*n+17, 2:65:2], start=False, stop=False)
                        nc.tensor.matmul(
                            a[:, :], whi[0:112, 3*384 + 128*m:3*384 + 128*m + 128],
                            skv[0:112, 16*n:16*n+16, 0:63:2], start=False, stop=True)
                st["a1"] = a1

            def epi1(s):
                st = state[s]
                self_affine(nc, tpool, epool, mpool, st["a1"], eb, 0, st["x2h"],
                            st["y2v"], y2_mode=True)
                # contiguous copies of y2 interior for the output scatter,
                # then scatter y2's channels now (off the critical path).
                y2c = [ycpool.tile([96, 1024], BF16, tag=f"y2c_{dj}", name=f"y2c_{s}_{dj}")
                       for dj in range(2)]
                for dj in range(2):
                    nc.vector.tensor_copy(
                        y2c[dj].rearrange("p (i w) -> p i w", i=32),
                        st["y2v"][dj][0:96, 1:33, 1:33])
                    nc.gpsimd.indirect_dma_start(
                        out=p_out[:],
                        out_offset=bass.IndirectOffsetOnAxis(
                            ap=tix[:, (2+dj)*SPS+s:(2+dj)*SPS+s+1], axis=0),
                        in_=y2c[dj][:, :], in_offset=None)

            def build_T(s):
                # Replica fills via SBUF->SBUF DMA (engine copies crawl under
                # SBUF port contention with the matmul stream).  T3 rows 0:64
                # (sigma=1) are stored shifted one column right so a single
                # col-base-2 AP works.
                T1, T2, T3 = stk[s % 2]
                nc.sync.dma_start(out=T1[96:128, :], in_=T2[0:32, :])
                nc.sync.dma_start(out=T2[96:128, :], in_=T1[64:96, :])
                nc.sync.dma_start(out=T3[0:64, 1:34*34], in_=T2[32:96, 0:34*34-1])
                nc.sync.dma_start(out=T3[64:128, :], in_=T1[0:64, :])

            def conv_lo(s):
                st = state[s]
                a2 = [[ps.tile([128, NCHUNK], F32, tag="psum", name=f"a2_{s}_{m}_{n}")
                       for n in range(2)] for m in range(3)]
                stkv = st["stkv"]
                # (view, K, col base); taps t*3+ki match lhsT_lo layout
                tiles = [(stkv[0], 96, 0), (stkv[1], 96, 0),
                         (stkv[0], 128, 1), (stkv[1], 128, 2), (stkv[2], 128, 2)]
                for n in range(2):
                    for m in range(3):
                        idx = 0
                        for t, (view, K, cb) in enumerate(tiles):
                            for ki in range(3):
                                rhs = view[0:K, 16*n+ki:16*n+ki+16, cb:cb+32]
                                nc.tensor.matmul(
                                    a2[m][n][:, :],
                                    wlo[0:K, (t*3+ki)*384 + 128*m:(t*3+ki)*384 + 128*m + 128],
                                    rhs, start=(idx == 0), stop=(idx == 14))
                                idx += 1
                st["a2"] = a2

            def epi2(s):
                st = state[s]
                y1t = [mpool.tile([96, 1024], BF16, tag=f"y1_{dj}", name=f"y1_{s}_{dj}")
                       for dj in range(2)]
                y1v = [y1t[dj].rearrange("p (n f) -> p n f", n=2) for dj in range(2)]

                def scat(t):
                    nc.gpsimd.indirect_dma_start(
                        out=p_out[:],
                        out_offset=bass.IndirectOffsetOnAxis(
                            ap=tix[:, t*SPS+s:t*SPS+s+1], axis=0),
                        in_=y1t[t][:, :], in_offset=None)

                self_affine(nc, tpool, epool, mpool, st["a2"], eb, 2, st["x1h"],
                            y1v, y2_mode=False, scatter_cb=scat)
                st["y1t"] = y1t

            load_dma(0, defer_u=True); sk_copy(0, split=True)
            conv_hi(0)
            # y2 borders in T1/T2 rows 0:96 zeroed (epi1 writes interior only);
            # memset on-engine beats 0.9MB of zero DMAs in the critical prefix
            for sl in range(2):
                for k in range(2):
                    eng = nc.gpsimd if k == 0 else nc.vector
                    eng.memset(stk[sl][k][0:96, :], 0)
            nc.sync.dma_start(out=eb, in_=p_eb[:])
            nc.sync.dma_start(out=tix, in_=p_idx[:])
            nc.sync.dma_start(out=state[0]["x1h_f"], in_=p_x1h[0])
            nc.sync.dma_start(out=state[0]["x2h_f"], in_=p_x2h[0])
            load_dma(1)
            # wlo split: sigma=0 taps (first 6 matmuls of each chunk) first,
            # the rest lands behind the build_T(0) replica fills
            nc.sync.dma_start(out=wlo[:, 0:6*384], in_=p_wlo[:, 0:6*384])
            epi1(0); build_T(0)
            nc.sync.dma_start(out=wlo[:, 6*384:], in_=p_wlo[:, 6*384:])
            for s in range(SPS):
                if s + 1 < SPS:
                    if s >= 1:
                        load_dma(s + 1)
                    sk_copy(s + 1)
                    conv_hi(s + 1); epi1(s + 1); build_T(s + 1)
                conv_lo(s); epi2(s)
    return nc


def self_affine(nc, tpool, epool, mpool, a, eb, eb_base, xh_v, yv, y2_mode,
                scatter_cb=None):
    """y[dj] = u[dj] * exp(2*tanh(0.2*a_s) + ln_scale) + a_t.

    a: 3x2 PSUM tiles [128, 512] with rows ordered
       [E0(96), T0(0:32) | T0(32:96), E1(0:64) | T1(96), E1(64:96)].

    y2_mode: output rows are the padded y2 interior, and the n-chunks are
    split asymmetrically: n0 ops produce output rows oh 0-15; a small
    fragment then produces row oh=16 (the first row of the n1 PSUM chunk) so
    conv_lo's n0 half can start before the n1 bulk (rows 17-31) is done."""
    t = [tpool.tile([96, 1024], F32, tag=f"t{eb_base}_{dj}", name=f"t{eb_base}_{dj}") for dj in range(2)]
    e = [epool.tile([96, 1024], BF16, tag=f"e{eb_base}_{dj}", name=f"e{eb_base}_{dj}") for dj in range(2)]
    rr = lambda ap, i=16: ap.rearrange("p (i w) -> p i w", i=i)

    def tanh_exp(n, cs=slice(0, 512)):
        # dj=0 needs only the m0 chunk: emit its exp before the m1/m2 tanhs
        # so the dj=0 chain never queues behind later matmul chunks
        ns = slice(n * 512 + cs.start, n * 512 + cs.stop)
        ebs = lambda dj: eb[0:96, eb_base + dj:eb_base + dj + 1]
        nc.scalar.activation(t[0][0:96, ns], a[0][n][0:96, cs], AF.Tanh, scale=0.2)
        nc.scalar.activation(e[0][0:96, ns], t[0][0:96, ns], AF.Exp, bias=ebs(0), scale=2.0)
        nc.scalar.activation(t[1][0:64, ns], a[1][n][64:128, cs], AF.Tanh, scale=0.2)
        nc.scalar.activation(t[1][64:96, ns], a[2][n][96:128, cs], AF.Tanh, scale=0.2)
        nc.scalar.activation(e[1][0:96, ns], t[1][0:96, ns], AF.Exp, bias=ebs(1), scale=2.0)

    def mul_add(dj, n, dst, i0, i1, csl):
        # tmp = u * E over output rows oh in [16n+i0, 16n+i1)
        nr = i1 - i0
        tmp = mpool.tile([96, 512], BF16, tag=f"tmp{eb_base}_{dj}", name=f"tmp{eb_base}_{dj}")
        uview = xh_v[0:96, 16*n+i0:16*n+i1, dj:dj+63:2]
        ev = rr(e[dj][0:96, n*512 + i0*32:n*512 + i1*32], nr)
        tmv = rr(tmp[0:96, 0:nr*32], nr)
        nc.gpsimd.tensor_tensor(out=tmv[0:96], in0=uview, in1=ev, op=MUL)
        tv = rr(tmp[0:96, 0:nr*32], nr)
        pr = lambda ap: rr(ap[:, csl], nr)
        if dj == 0:
            nc.vector.tensor_tensor(out=dst[0:32], in0=tv[0:32], in1=pr(a[0][n][96:128, :]), op=ADD)
            nc.vector.tensor_tensor(out=dst[32:64], in0=tv[32:64], in1=pr(a[1][n][0:32, :]), op=ADD)
            nc.vector.tensor_tensor(out=dst[64:96], in0=tv[64:96], in1=pr(a[1][n][32:64, :]), op=ADD)
        else:
            nc.vector.tensor_tensor(out=dst[0:96], in0=tv[0:96], in1=pr(a[2][n][0:96, :]), op=ADD)

    # ---- n = 0: full 16 rows ----
    tanh_exp(0)
    for dj in range(2):
        if y2_mode:
            dst = yv[dj][0:96, 1:17, 1:33]
        else:
            dst = rr(yv[dj][0:96, 0, :])
        mul_add(dj, 0, dst, 0, 16, slice(0, 512))

    if not y2_mode:
        tanh_exp(1)
        for dj in range(2):
            mul_add(dj, 1, rr(yv[dj][0:96, 1, :]), 0, 16, slice(0, 512))
        # scat(0) only waits on the dj=0 chain (complete before the last
        # matmul chunk); its issue overlaps the dj=1 adds
        if scatter_cb is not None:
            scatter_cb(0); scatter_cb(1)
        return

    # ---- fragment: output row oh=16 (first col-block of the n1 chunks) ----
    fsl = slice(512, 544)
    nc.scalar.activation(t[0][0:96, fsl], a[0][1][0:96, 0:32], AF.Tanh, scale=0.2)
    nc.scalar.activation(t[1][0:64, fsl], a[1][1][64:128, 0:32], AF.Tanh, scale=0.2)
    nc.scalar.activation(t[1][64:96, fsl], a[2][1][96:128, 0:32], AF.Tanh, scale=0.2)
    for dj in range(2):
        nc.scalar.activation(e[dj][0:96, fsl], t[dj][0:96, fsl], AF.Exp,
                             bias=eb[0:96, eb_base + dj:eb_base + dj + 1], scale=2.0)
        ftmp = mpool.tile([96, 32], BF16, tag=f"ftmp_{dj}", name=f"ftmp_{dj}")
        uview = xh_v[0:96, 16:17, dj:dj+63:2]
        nc.gpsimd.tensor_tensor(out=ftmp[0:96, :].rearrange("p (i w) -> p i w", i=1),
                                in0=uview,
                                in1=e[dj][0:96, fsl].rearrange("p (i w) -> p i w", i=1), op=MUL)
        dst = yv[dj][0:96, 17:18, 1:33]
        fv = ftmp[0:96, :].rearrange("p (i w) -> p i w", i=1)
        pr = lambda ap: ap[:, 0:32].rearrange("p (i w) -> p i w", i=1)
        if dj == 0:
            nc.vector.tensor_tensor(out=dst[0:32], in0=fv[0:32], in1=pr(a[0][1][96:128, :]), op=ADD)
            nc.vector.tensor_tensor(out=dst[32:64], in0=fv[32:64], in1=pr(a[1][1][0:32, :]), op=ADD)
            nc.vector.tensor_tensor(out=dst[64:96], in0=fv[64:96], in1=pr(a[1][1][32:64, :]), op=ADD)
        else:
            nc.vector.tensor_tensor(out=dst[0:96], in0=fv[0:96], in1=pr(a[2][1][0:96, :]), op=ADD)

    # ---- n = 1 bulk: rows oh 17-31 (output rows 18-33) ----
    tanh_exp(1)
    for dj in range(2):
        dst = yv[dj][0:96, 18:33, 1:33]
        mul_add(dj, 1, dst, 1, 16, slice(32, 512))


# ---------------------------------------------------------------------------
# entry point
# ---------------------------------------------------------------------------
_CACHE = {}

def _get_nc():
    if "nc" not in _CACHE:
        nc = build_kernel()
        _split_excess_waits(nc)
        _CACHE["nc"] = nc
    return _CACHE["nc"]


def _split_excess_waits(nc):
    """This walrus allows 1 sync wait per instruction (2 for EventSemaphore);
    hoist overflow waits into standalone EventSemaphore insts."""
    import bass_rust
    n_fix = 0
    for fn in nc.m.functions:
        for blk in fn.blocks:
            insts = blk.instructions
            out = []
            for inst in insts:
                si = inst.sync_info
                cap = 2 if inst.opcode == "EventSemaphore" else 1
                if si is not None and len(si.on_wait) > cap:
                    waits = list(si.on_wait)
                    keep, extra = waits[:cap], waits[cap:]
                    for ci in range(0, len(extra), 2):
                        n_fix += 1
                        ev = mybir.InstEventSemaphore(name=f"I-waitfix-{n_fix}", ins=[], outs=[])
                        ev.engine = inst.engine
                        ev.sync_info = bass_rust.SyncInfo(on_wait=extra[ci:ci+2], on_update=[])
                        nc.register_instruction(ev, overwrite=True)
                        out.append(ev)
                    inst.sync_info = bass_rust.SyncInfo(on_wait=keep, on_update=list(si.on_update))
                out.append(inst)
            if len(out) != len(insts):
                blk.instructions = out
    return n_fix


def _install_profile_shim():
    import types
    name = "antenv.axon_hooks"
    if name in sys.modules:
        return
    try:
        from trn_agent_boot.trn_boot import _ntff_profile_via_ctypes
        hook = _ntff_profile_via_ctypes('/opt/axon/libaxon_pjrt.so')
    except Exception:
        hook = None
    mod = types.ModuleType(name)
    mod._hook = hook
    mod.get_axon_ntff_profile_hook = lambda: mod._hook
    mod.set_axon_ntff_profile_hook = lambda h: setattr(mod, '_hook', h)
    sys.modules[name] = mod


def run(inputs, trace=False):
    _install_profile_shim()
    per_core = _prepare(**inputs)
    nc = _get_nc()
    res = run_bass_kernel_spmd(nc, per_core, core_ids=list(range(N_CORES)), trace=trace)
    outs = [res.results[i]["out"].reshape(SPS, 384, 32, 32) for i in range(N_CORES)]
    full = np.concatenate(outs, axis=0).astype(np.float32)
    return full, res


def kernel(**inputs):
    full, _ = run(inputs, trace=False)
    return full
